# revision 3
# baseline (speedup 1.0000x reference)
"""MultiHeadAttention kernel for Trainium2, 8-core hybrid batch x head sharding.

Problem: S=2048, B=2, D=1024, 16 heads of d=64 (batch_first=False).
Sharding: core c handles batch b=c//4 and head group hg=c%4 (4 heads =
256 output dims), processed as 2 "pairs" of 2 heads (a pair = 128
partitions = 2x64 head dims). Each core reads only its batch's
activations (12MB instead of 24MB) plus its 256-column weight slices.

Per-core dataflow:
  q^T, k^T [128, S] per pair = W_pair @ x^T            (PE, bf16, fp32 psum)
  v^T      [128, S] likewise -> DMA-xbar transpose to token-major
           v' [tok, jt, head, 65] with a ones column (softmax denom)
  scores   per j-tile: both heads into ONE [128, 1024] psum tile
           (h0 cols 0:512, h1 cols 512:1024), K=64 matmuls at row
           positions 0/64 (row-packable)
  attn     = exp(scores * 1/8) in ONE ScalarE activation [128, 1024]
           (no max-subtract: scores*scale is small), bf16 out
  pv       [65, 512] per head += v'^T . attn, trailing the exps by one
           j-tile so the PE never waits on the current exp
  out      = pv[0:64] / pv[64] via reciprocal on a [128, 8] reshape,
           ones-matmul broadcast (into the projection psum tag, NOT the
           live pv banks), DVE multiply
The exp stream is the critical engine (128 x [128,1024] activations
~= 128us); projections and v-transposes are deadline-scheduled into the
PE/DMA idle slack of the attention pipeline.
"""

import sys

if "/opt/trn_rl_repo" not in sys.path:
    sys.path.insert(0, "/opt/trn_rl_repo")

import numpy as np
import ml_dtypes

import concourse.bass as bass
import concourse.mybir as mybir
import concourse.tile as tile
from concourse import bacc

BF16 = mybir.dt.bfloat16
FP32 = mybir.dt.float32
NP_BF16 = ml_dtypes.bfloat16

D = 1024
NHEAD = 16
DH = 64
NCORES = 8
S = 2048
B = 2
HPC = 4                      # heads per core
DC = HPC * DH                # per-core output dims = 256
NPAIR = 2                    # head pairs per core (128 dims each)
KT = D // 128                # contraction tiles = 8
TB = 512                     # token block for projections
NTB = S // TB                # 4
IC = 512                     # i-chunk width
NIC = S // IC                # 4
JT = S // 128                # j-tiles = 16
SCALE = 1.0 / float(np.sqrt(DH))


def build_program():
    nc = bacc.Bacc(
        "TRN2", target_bir_lowering=False, debug=False, num_devices=NCORES
    )
    xq = nc.dram_tensor("xq", [NTB, 128, KT, TB], BF16, kind="ExternalInput")
    xk = nc.dram_tensor("xk", [NTB, 128, KT, TB], BF16, kind="ExternalInput")
    xv = nc.dram_tensor("xv", [NTB, 128, KT, TB], BF16, kind="ExternalInput")
    wq = nc.dram_tensor("wq", [128, KT, DC], BF16, kind="ExternalInput")
    wk = nc.dram_tensor("wk", [128, KT, DC], BF16, kind="ExternalInput")
    wv = nc.dram_tensor("wv", [128, KT, DC], BF16, kind="ExternalInput")
    bqkv = nc.dram_tensor("bqkv", [128, NPAIR, 3], FP32, kind="ExternalInput")
    out = nc.dram_tensor("out", [DC, S], FP32, kind="ExternalOutput")

    with tile.TileContext(nc) as tc:
        with (
            tc.tile_pool(name="const", bufs=1) as constp,
            tc.tile_pool(name="xin", bufs=1) as xp,
            tc.tile_pool(name="qkv", bufs=1) as qkvp,
            tc.tile_pool(name="vstg", bufs=2) as vstgp,
            tc.tile_pool(name="attn", bufs=2) as atp,
            tc.tile_pool(name="outp", bufs=2) as outp,
            tc.tile_pool(name="drain", bufs=2) as drainp,
            tc.tile_pool(name="sc", bufs=3, space="PSUM") as scp,
            tc.tile_pool(name="pv", bufs=2, space="PSUM") as pvp,
        ):
            # ---- input loads first, ALL on the SWDGE (gpsimd) queue which
            # starts draining earliest; interleaved so each projection's
            # weight arrives just before its x tile
            xq_t = xp.tile([128, NTB, KT, TB], BF16, tag="xq")
            xk_t = xp.tile([128, NTB, KT, TB], BF16, tag="xk")
            xv_t = xp.tile([128, NTB, KT, TB], BF16, tag="xv")
            xts = {0: xq_t, 1: xk_t, 2: xv_t}
            xsrc = {0: xq, 1: xk, 2: xv}
            wq_t = constp.tile([128, KT, DC], BF16, tag="wq")
            wk_t = constp.tile([128, KT, DC], BF16, tag="wk")
            wv_t = constp.tile([128, KT, DC], BF16, tag="wv")

            def dma_x(kind, tb):
                nc.gpsimd.dma_start(out=xts[kind][:, tb], in_=xsrc[kind][tb])

            nc.gpsimd.dma_start(out=wq_t[:], in_=wq[:, :, :])
            dma_x(0, 0)
            nc.gpsimd.dma_start(out=wk_t[:], in_=wk[:, :, :])
            dma_x(1, 0)
            nc.gpsimd.dma_start(out=wv_t[:], in_=wv[:, :, :])
            dma_x(2, 0)
            # the rest of the stream in consumption order; the queue drains
            # continuously at HBM rate, so emitting everything now is strictly
            # better than injecting starts later
            for kind, tb in [(1, 1), (2, 1), (1, 2), (2, 2), (1, 3), (2, 3),
                             (0, 1), (0, 2), (0, 3)]:
                dma_x(kind, tb)

            # ---- exp table preload: fire ACT_TABLE_LOAD during the DMA phase
            pre_in = constp.tile([128, 16], FP32, tag="prei")
            nc.vector.memset(pre_in[:], 0.0)
            pre_out = constp.tile([128, 16], BF16, tag="preo")
            nc.scalar.activation(
                out=pre_out[:], in_=pre_in[:],
                func=mybir.ActivationFunctionType.Exp, scale=1.0,
            )

            # ---- constants
            bqkv_t = constp.tile([128, NPAIR, 3], FP32, tag="bqkv")
            nc.sync.dma_start(out=bqkv_t[:], in_=bqkv[:, :, :])

            # ---- persistent activations
            q_t = qkvp.tile([128, NPAIR, S], BF16, tag="q")
            k_t = qkvp.tile([128, NPAIR, S], BF16, tag="k")
            # token-major projected v, two stages: vx_t is the contiguous
            # DMA-xbar transpose target [tok, jt, 128 dims]; v_t adds the
            # ones column per head (softmax denominator) via one cheap DVE
            # rearrange copy per projection unit.
            vx_t = qkvp.tile([128, NPAIR, JT, 2 * DH], BF16, tag="vx")
            # per head: cols 0:64 = projected v, cols 64:128 = ones, so the
            # pv matmul replicates the softmax denominator across output
            # partitions 64:128 (normalization then needs no partition
            # broadcast at all)
            v_t = qkvp.tile([128, NPAIR, JT, 2, 2 * DH], BF16, tag="v")
            nc.vector.memset(v_t[:, :, :, :, DH : 2 * DH], 1.0)

            wts = {0: wq_t, 1: wk_t, 2: wv_t}

            # ---- HAM warmup: a dense identical-weights matmul burst chained
            # on the weights so the PE monitor reaches 8/8 before projections
            warm = scp.tile([128, 128], FP32, tag="sc", name="warm")
            for _ in range(24):
                nc.tensor.matmul(
                    warm[:, :], wq_t[:, 0, 0:128], wq_t[:, 0, 0:128],
                    start=True, stop=True,
                )

            proj_ps = {}

            def emit_proj_mm(kind, tb, p, half):
                # one half of a projection's K-accumulation: 4 matmuls ~0.9us
                x_t = xts[kind]
                w_t = wts[kind]
                if half == 0:
                    # projection scratch shares the (triple-buffered) scores
                    # tag: a proj ps occupies one slot for ~2 points while
                    # scores keep double-buffering through the other two
                    proj_ps[(kind, tb, p)] = scp.tile(
                        [128, TB], FP32, tag="sc", name=f"ps{kind}{tb}{p}"
                    )
                ps = proj_ps[(kind, tb, p)]
                for kt in range(half * 4, half * 4 + 4):
                    nc.tensor.matmul(
                        ps[:, :], w_t[:, kt, p * 128 : (p + 1) * 128],
                        x_t[:, tb, kt, :],
                        start=(kt == 0), stop=(kt == KT - 1),
                    )
                if half == 1:
                    bias = bqkv_t[:, p, kind : kind + 1].to_broadcast((128, TB))
                    if kind < 2:
                        dst = q_t if kind == 0 else k_t
                        nc.vector.tensor_add(
                            dst[:, p, tb * TB : (tb + 1) * TB], ps[:, :], bias
                        )
                    else:
                        vTt = vstgp.tile([128, TB], BF16, tag="vT", name="vT")
                        nc.vector.tensor_add(vTt[:, :], ps[:, :], bias)
                        j0 = tb * (TB // 128)
                        j1 = (tb + 1) * (TB // 128)
                        nc.sync.dma_start_transpose(vx_t[:, p, j0:j1, :], vTt[:, :])
                        nc.vector.tensor_copy(
                            v_t[:, p, j0:j1, :, 0:DH],
                            vx_t[:, p, j0:j1, :].rearrange("t j (h d) -> t j h d", h=2),
                        )
                    del proj_ps[(kind, tb, p)]

            def emit_proj(kind, tb, p):
                emit_proj_mm(kind, tb, p, 0)
                emit_proj_mm(kind, tb, p, 1)

            # prologue: just enough for p0's chunk-0 to start (j-tiles 0-3
            # only need k/v block 0); the rest injects at deadline points
            emit_proj(0, 0, 0)
            emit_proj(1, 0, 0)
            emit_proj(2, 0, 0)

            # ---- deadline-scheduled units (half-projection granularity so no
            # single injection holds the PE for long).
            # point = (p*NIC + ic)*JT + jt.
            # p0 deadlines (finish by): k(tb) 4*tb, v(tb) 4*tb+1, q(tb) 16*tb.
            # p1 deadlines: +64 on everything.
            def _scoped(name, fn):
                def g():
                    with nc.named_scope(name):
                        fn()
                return g

            def proj_halves(pt0, pt1, kind, tb, p):
                n = f"U{'qkv'[kind]}{tb}p{p}"
                return [
                    (pt0, _scoped(n + "a", lambda: emit_proj_mm(kind, tb, p, 0))),
                    (pt1, _scoped(n + "b", lambda: emit_proj_mm(kind, tb, p, 1))),
                ]

            units = (
                proj_halves(1, 2, 1, 1, 0)
                + proj_halves(2, 3, 2, 1, 0)
                + proj_halves(4, 5, 1, 2, 0)
                + proj_halves(6, 7, 2, 2, 0)
                + proj_halves(8, 9, 1, 3, 0)
                + proj_halves(10, 11, 2, 3, 0)
                + proj_halves(13, 14, 0, 1, 0)
                + proj_halves(18, 20, 0, 2, 0)
                + proj_halves(24, 26, 0, 3, 0)
                + proj_halves(28, 30, 1, 0, 1)
                + proj_halves(32, 34, 2, 0, 1)
                + proj_halves(36, 38, 1, 1, 1)
                + proj_halves(40, 42, 2, 1, 1)
                + proj_halves(44, 46, 1, 2, 1)
                + proj_halves(48, 50, 2, 2, 1)
                + proj_halves(52, 54, 1, 3, 1)
                + proj_halves(56, 58, 2, 3, 1)
                + proj_halves(60, 62, 0, 0, 1)
                + proj_halves(68, 70, 0, 1, 1)
                + proj_halves(84, 86, 0, 2, 1)
                + proj_halves(100, 102, 0, 3, 1)
            )
            units.sort(key=lambda u: u[0])
            ui = [0]
            pending_steps = []

            def inject(point):
                while ui[0] < len(units) and units[ui[0]][0] <= point:
                    units[ui[0]][1]()
                    ui[0] += 1
                if pending_steps:
                    pending_steps.pop(0)()

            def emit_attention(p):
                for ic in range(NIC):
                    i0 = ic * IC
                    pv = [
                        pvp.tile([128, IC], FP32, tag="pv", name=f"pv{h}")
                        for h in range(2)
                    ]
                    at = atp.tile([128, 6, 2 * IC], BF16, tag="at", name="at")

                    def emit_pv(j):
                        for h in range(2):
                            nc.tensor.matmul(
                                pv[h][:, :],
                                v_t[:, p, j, h, :],
                                at[:, j % 6, h * IC : (h + 1) * IC],
                                start=(j == 0), stop=(j == JT - 1),
                            )

                    for jt in range(JT):
                        # scores + exp FIRST: they are the ACT-critical chain,
                        # and the PE queue is in-order — anything emitted
                        # before sc(jt) delays the next exp by its duration
                        _pt = (p * NIC + ic) * JT + jt
                        # high_priority: the scores->exp chain is the critical
                        # path; this lets a ready score matmul preempt queued
                        # projection/finalize work in the scheduler's heap
                        with nc.named_scope(f"S{_pt}"):
                            sc = scp.tile([128, 2 * IC], FP32, tag="sc", name="sc")
                            for h in range(2):
                                nc.tensor.matmul(
                                    sc[:, h * IC : (h + 1) * IC],
                                    k_t[h * DH : (h + 1) * DH, p, jt * 128 : (jt + 1) * 128],
                                    q_t[h * DH : (h + 1) * DH, p, i0 : i0 + IC],
                                    start=True, stop=True,
                                )
                            nc.scalar.activation(
                                out=at[:, jt % 6, :], in_=sc[:, :],
                                func=mybir.ActivationFunctionType.Exp,
                                scale=SCALE,
                            )
                        inject(_pt)
                        if jt >= 1:
                            with nc.named_scope(f"P{_pt}"):
                                emit_pv(jt - 1)
                    with nc.named_scope(f"Pend{p}_{ic}"):
                        emit_pv(JT - 1)

                    # normalization: denominator sits replicated in pv
                    # rows 64:128, so 1/den is one fast DVE op and the
                    # multiply doubles as the psum evacuation — no partition
                    # broadcast, no PE involvement, nothing on the exp path
                    for h in range(2):
                        densb = drainp.tile([DH, IC], FP32, tag="densb", name="densb")
                        nc.vector.tensor_copy(densb[:, :], pv[h][DH : 2 * DH, :])
                        rec = drainp.tile([DH, IC], FP32, tag="rec", name="rec")
                        nc.vector.reciprocal_approx_fast(rec[:, :], densb[:, :])
                        osb = outp.tile([DH, IC], FP32, tag="osb", name="osb")
                        nc.vector.tensor_mul(osb[:, :], pv[h][0:DH, :], rec[:, :])
                        nc.sync.dma_start(
                            out=out[
                                (2 * p + h) * DH : (2 * p + h + 1) * DH,
                                i0 : i0 + IC,
                            ],
                            in_=osb[:, :],
                        )

            emit_attention(0)
            emit_attention(1)
            for fn in pending_steps:
                fn()
            pending_steps.clear()

    nc.finalize()
    return nc


_PROGRAM_CACHE = {}


def _get_program(S_, B_):
    assert (S_, B_) == (S, B)
    if "p" not in _PROGRAM_CACHE:
        _PROGRAM_CACHE["p"] = build_program()
    return _PROGRAM_CACHE["p"]


def make_in_maps(query, key, value, Wq, bq, Wk, bk, Wv, bv):
    S_, B_, D_ = query.shape
    assert (S_, B_, D_) == (S, B, D)

    def xt(a, b):
        # [S, B, D] -> [D, S] for batch b -> tiles [NTB, 128, KT, TB]
        aT = np.asarray(a[:, b, :], np.float32).T
        a4 = aT.reshape(KT, 128, NTB, TB).transpose(2, 1, 0, 3)
        return np.ascontiguousarray(a4).astype(NP_BF16)

    def wt_host(W, rows):
        # [DC rows, D] slice -> W.T [D, DC] -> [128, KT, DC] (partition-major)
        wT = np.asarray(W)[rows, :].T.astype(np.float32)
        w3 = wT.reshape(KT, 128, DC).transpose(1, 0, 2)
        return np.ascontiguousarray(w3).astype(NP_BF16)

    xq_b = [xt(query, b) for b in range(B)]
    xk_b = [xt(key, b) for b in range(B)]
    xv_b = [xt(value, b) for b in range(B)]

    in_maps = []
    for c in range(NCORES):
        b, hg = c // 4, c % 4
        rows = slice(hg * DC, (hg + 1) * DC)
        in_maps.append(
            {
                "xq": xq_b[b], "xk": xk_b[b], "xv": xv_b[b],
                "wq": wt_host(Wq, rows),
                "wk": wt_host(Wk, rows),
                "wv": wt_host(Wv, rows),
                "bqkv": np.ascontiguousarray(
                    np.stack(
                        [np.asarray(bq)[rows], np.asarray(bk)[rows], np.asarray(bv)[rows]],
                        axis=1,
                    ).reshape(NPAIR, 128, 3).transpose(1, 0, 2)
                ).astype(np.float32),
            }
        )
    return in_maps


def gather_output(results, S_, B_):
    full = np.empty((S, B, D), np.float32)
    for c in range(NCORES):
        b, hg = c // 4, c % 4
        o = np.asarray(results[c]["out"], np.float32)  # [DC, S]
        full[:, b, hg * DC : (hg + 1) * DC] = o.T
    return full


def kernel(query, key, value, Wq, bq, Wk, bk, Wv, bv):
    from concourse.bass_utils import run_bass_kernel_spmd

    S_, B_, _ = query.shape
    nc = _get_program(S_, B_)
    in_maps = make_in_maps(query, key, value, Wq, bq, Wk, bk, Wv, bv)
    res = run_bass_kernel_spmd(nc, in_maps, list(range(NCORES)))
    return gather_output(res.results, S_, B_)


# revision 4
# speedup vs baseline: 1.0172x; 1.0172x over previous
"""MultiHeadAttention kernel for Trainium2, 8-core hybrid batch x head sharding.

Problem: S=2048, B=2, D=1024, 16 heads of d=64 (batch_first=False).
Sharding: core c handles batch b=c//4 and head group hg=c%4 (4 heads =
256 output dims), processed as 2 "pairs" of 2 heads (a pair = 128
partitions = 2x64 head dims). Each core reads only its batch's
activations (12MB instead of 24MB) plus its 256-column weight slices.

Per-core dataflow:
  q^T, k^T [128, S] per pair = W_pair @ x^T            (PE, bf16, fp32 psum)
  v^T      [128, S] likewise -> DMA-xbar transpose to token-major
           v' [tok, jt, head, 128] with 64 ones columns (softmax denom)
  scores   per j-tile: both heads into ONE [128, 1024] psum tile
           (h0 cols 0:512, h1 cols 512:1024), K=64 matmuls at row
           positions 0/64 (row-packable)
  attn     = exp(scores * 1/8) in ONE ScalarE activation [128, 1024]
           (no max-subtract: scores*scale is small), bf16 out
  pv       [128, 512] per head += v'^T . attn, trailing the exps by one
           j-tile; v' cols 64:128 are ones, so pv rows 64:128 hold the
           softmax denominator replicated across partitions
  out      = pv[0:64] * recip(pv[64:128]): one SBUF staging copy, one
           reciprocal_approx_fast, one DVE multiply — no partition
           broadcast, no DMA round-trips, nothing on the PE
The exp stream is the critical engine (128 x [128,1024] activations
~= 128us); projections and v-transposes are deadline-scheduled into the
PE/DMA idle slack of the attention pipeline.
"""

import sys

if "/opt/trn_rl_repo" not in sys.path:
    sys.path.insert(0, "/opt/trn_rl_repo")

import numpy as np
import ml_dtypes

import concourse.bass as bass
import concourse.mybir as mybir
import concourse.tile as tile
from concourse import bacc

BF16 = mybir.dt.bfloat16
FP32 = mybir.dt.float32
NP_BF16 = ml_dtypes.bfloat16

D = 1024
NHEAD = 16
DH = 64
NCORES = 8
S = 2048
B = 2
HPC = 4                      # heads per core
DC = HPC * DH                # per-core output dims = 256
NPAIR = 2                    # head pairs per core (128 dims each)
KT = D // 128                # contraction tiles = 8
TB = 512                     # token block for projections
NTB = S // TB                # 4
IC = 512                     # i-chunk width
NIC = S // IC                # 4
JT = S // 128                # j-tiles = 16
SCALE = 1.0 / float(np.sqrt(DH))


def build_program():
    nc = bacc.Bacc(
        "TRN2", target_bir_lowering=False, debug=False, num_devices=NCORES
    )
    xq = nc.dram_tensor("xq", [NTB, 128, KT, TB], BF16, kind="ExternalInput")
    xk = nc.dram_tensor("xk", [NTB, 128, KT, TB], BF16, kind="ExternalInput")
    xv = nc.dram_tensor("xv", [NTB, 128, KT, TB], BF16, kind="ExternalInput")
    wq = nc.dram_tensor("wq", [128, KT, DC], BF16, kind="ExternalInput")
    wk = nc.dram_tensor("wk", [128, KT, DC], BF16, kind="ExternalInput")
    wv = nc.dram_tensor("wv", [128, KT, DC], BF16, kind="ExternalInput")
    bqkv = nc.dram_tensor("bqkv", [128, NPAIR, 3], FP32, kind="ExternalInput")
    out = nc.dram_tensor("out", [DC, S], FP32, kind="ExternalOutput")

    with tile.TileContext(nc) as tc:
        with (
            tc.tile_pool(name="const", bufs=1) as constp,
            tc.tile_pool(name="xin", bufs=1) as xp,
            tc.tile_pool(name="qkv", bufs=1) as qkvp,
            tc.tile_pool(name="vstg", bufs=2) as vstgp,
            tc.tile_pool(name="attn", bufs=2) as atp,
            tc.tile_pool(name="outp", bufs=2) as outp,
            tc.tile_pool(name="drain", bufs=2) as drainp,
            tc.tile_pool(name="sc", bufs=3, space="PSUM") as scp,
            tc.tile_pool(name="pv", bufs=2, space="PSUM") as pvp,
        ):
            # ---- input loads first, ALL on the SWDGE (gpsimd) queue which
            # starts draining earliest; interleaved so each projection's
            # weight arrives just before its x tile
            xq_t = xp.tile([128, NTB, KT, TB], BF16, tag="xq")
            xk_t = xp.tile([128, NTB, KT, TB], BF16, tag="xk")
            xv_t = xp.tile([128, NTB, KT, TB], BF16, tag="xv")
            xts = {0: xq_t, 1: xk_t, 2: xv_t}
            xsrc = {0: xq, 1: xk, 2: xv}
            wq_t = constp.tile([128, KT, DC], BF16, tag="wq")
            wk_t = constp.tile([128, KT, DC], BF16, tag="wk")
            wv_t = constp.tile([128, KT, DC], BF16, tag="wv")

            def dma_x(kind, tb):
                nc.gpsimd.dma_start(out=xts[kind][:, tb], in_=xsrc[kind][tb])

            nc.gpsimd.dma_start(out=wq_t[:], in_=wq[:, :, :])
            dma_x(0, 0)
            nc.gpsimd.dma_start(out=wk_t[:], in_=wk[:, :, :])
            dma_x(1, 0)
            nc.gpsimd.dma_start(out=wv_t[:], in_=wv[:, :, :])
            dma_x(2, 0)
            # the rest of the stream in consumption order; the queue drains
            # continuously at HBM rate, so emitting everything now is strictly
            # better than injecting starts later
            for kind, tb in [(1, 1), (2, 1), (1, 2), (2, 2), (1, 3), (2, 3),
                             (0, 1), (0, 2), (0, 3)]:
                dma_x(kind, tb)

            # ---- exp table preload: fire ACT_TABLE_LOAD during the DMA phase
            pre_in = constp.tile([128, 16], FP32, tag="prei")
            nc.vector.memset(pre_in[:], 0.0)
            pre_out = constp.tile([128, 16], BF16, tag="preo")
            nc.scalar.activation(
                out=pre_out[:], in_=pre_in[:],
                func=mybir.ActivationFunctionType.Exp, scale=1.0,
            )

            # ---- constants
            bqkv_t = constp.tile([128, NPAIR, 3], FP32, tag="bqkv")
            nc.sync.dma_start(out=bqkv_t[:], in_=bqkv[:, :, :])

            # ---- persistent activations
            q_t = qkvp.tile([128, NPAIR, S], BF16, tag="q")
            k_t = qkvp.tile([128, NPAIR, S], BF16, tag="k")
            # token-major projected v, two stages: vx_t is the contiguous
            # DMA-xbar transpose target [tok, jt, 128 dims]; v_t adds the
            # ones column per head (softmax denominator) via one cheap DVE
            # rearrange copy per projection unit.
            vx_t = qkvp.tile([128, NPAIR, JT, 2 * DH], BF16, tag="vx")
            # per head: cols 0:64 = projected v, cols 64:128 = ones, so the
            # pv matmul replicates the softmax denominator across output
            # partitions 64:128 (normalization then needs no partition
            # broadcast at all)
            v_t = qkvp.tile([128, NPAIR, JT, 2, 2 * DH], BF16, tag="v")
            nc.vector.memset(v_t[:, :, :, :, DH : 2 * DH], 1.0)

            wts = {0: wq_t, 1: wk_t, 2: wv_t}

            # ---- HAM warmup: a dense identical-weights matmul burst chained
            # on the weights so the PE monitor reaches 8/8 before projections
            warm = scp.tile([128, 128], FP32, tag="sc", name="warm")
            for _ in range(24):
                nc.tensor.matmul(
                    warm[:, :], wq_t[:, 0, 0:128], wq_t[:, 0, 0:128],
                    start=True, stop=True,
                )

            proj_ps = {}

            def emit_proj_mm(kind, tb, p, half):
                # one half of a projection's K-accumulation: 4 matmuls ~0.9us
                x_t = xts[kind]
                w_t = wts[kind]
                if half == 0:
                    # projection scratch shares the (triple-buffered) scores
                    # tag: a proj ps occupies one slot for ~2 points while
                    # scores keep double-buffering through the other two
                    proj_ps[(kind, tb, p)] = scp.tile(
                        [128, TB], FP32, tag="sc", name=f"ps{kind}{tb}{p}"
                    )
                ps = proj_ps[(kind, tb, p)]
                for kt in range(half * 4, half * 4 + 4):
                    nc.tensor.matmul(
                        ps[:, :], w_t[:, kt, p * 128 : (p + 1) * 128],
                        x_t[:, tb, kt, :],
                        start=(kt == 0), stop=(kt == KT - 1),
                    )
                if half == 1:
                    bias = bqkv_t[:, p, kind : kind + 1].to_broadcast((128, TB))
                    if kind < 2:
                        dst = q_t if kind == 0 else k_t
                        nc.vector.tensor_add(
                            dst[:, p, tb * TB : (tb + 1) * TB], ps[:, :], bias
                        )
                    else:
                        vTt = vstgp.tile([128, TB], BF16, tag="vT", name="vT")
                        nc.vector.tensor_add(vTt[:, :], ps[:, :], bias)
                        j0 = tb * (TB // 128)
                        j1 = (tb + 1) * (TB // 128)
                        nc.sync.dma_start_transpose(vx_t[:, p, j0:j1, :], vTt[:, :])
                        nc.vector.tensor_copy(
                            v_t[:, p, j0:j1, :, 0:DH],
                            vx_t[:, p, j0:j1, :].rearrange("t j (h d) -> t j h d", h=2),
                        )
                    del proj_ps[(kind, tb, p)]

            def emit_proj(kind, tb, p):
                emit_proj_mm(kind, tb, p, 0)
                emit_proj_mm(kind, tb, p, 1)

            # prologue: just enough for p0's chunk-0 to start (j-tiles 0-3
            # only need k/v block 0); the rest injects at deadline points
            emit_proj(0, 0, 0)
            emit_proj(1, 0, 0)
            emit_proj(2, 0, 0)

            # ---- deadline-scheduled units (half-projection granularity so no
            # single injection holds the PE for long).
            # point = (p*NIC + ic)*JT + jt.
            # p0 deadlines (finish by): k(tb) 4*tb, v(tb) 4*tb+1, q(tb) 16*tb.
            # p1 deadlines: +64 on everything.
            def _scoped(name, fn):
                def g():
                    with nc.named_scope(name):
                        fn()
                return g

            def proj_halves(pt0, pt1, kind, tb, p):
                n = f"U{'qkv'[kind]}{tb}p{p}"
                return [
                    (pt0, _scoped(n + "a", lambda: emit_proj_mm(kind, tb, p, 0))),
                    (pt1, _scoped(n + "b", lambda: emit_proj_mm(kind, tb, p, 1))),
                ]

            units = (
                proj_halves(1, 2, 1, 1, 0)
                + proj_halves(2, 3, 2, 1, 0)
                + proj_halves(4, 5, 1, 2, 0)
                + proj_halves(6, 7, 2, 2, 0)
                + proj_halves(8, 9, 1, 3, 0)
                + proj_halves(10, 11, 2, 3, 0)
                + proj_halves(13, 14, 0, 1, 0)
                + proj_halves(18, 20, 0, 2, 0)
                + proj_halves(24, 26, 0, 3, 0)
                + proj_halves(28, 30, 1, 0, 1)
                + proj_halves(32, 34, 2, 0, 1)
                + proj_halves(36, 38, 1, 1, 1)
                + proj_halves(40, 42, 2, 1, 1)
                + proj_halves(44, 46, 1, 2, 1)
                + proj_halves(48, 50, 2, 2, 1)
                + proj_halves(52, 54, 1, 3, 1)
                + proj_halves(56, 58, 2, 3, 1)
                + proj_halves(60, 62, 0, 0, 1)
                + proj_halves(68, 70, 0, 1, 1)
                + proj_halves(84, 86, 0, 2, 1)
                + proj_halves(100, 102, 0, 3, 1)
            )
            units.sort(key=lambda u: u[0])
            ui = [0]
            pending_steps = []

            def inject(point):
                while ui[0] < len(units) and units[ui[0]][0] <= point:
                    units[ui[0]][1]()
                    ui[0] += 1
                if pending_steps:
                    pending_steps.pop(0)()

            def emit_attention(p):
                for ic in range(NIC):
                    i0 = ic * IC
                    pv = [
                        pvp.tile([128, IC], FP32, tag="pv", name=f"pv{h}")
                        for h in range(2)
                    ]
                    at = atp.tile([128, 6, 2 * IC], BF16, tag="at", name="at")

                    def emit_pv(j):
                        for h in range(2):
                            nc.tensor.matmul(
                                pv[h][:, :],
                                v_t[:, p, j, h, :],
                                at[:, j % 6, h * IC : (h + 1) * IC],
                                start=(j == 0), stop=(j == JT - 1),
                            )

                    for jt in range(JT):
                        # scores + exp FIRST: they are the ACT-critical chain,
                        # and the PE queue is in-order — anything emitted
                        # before sc(jt) delays the next exp by its duration
                        _pt = (p * NIC + ic) * JT + jt
                        # high_priority: the scores->exp chain is the critical
                        # path; this lets a ready score matmul preempt queued
                        # projection/finalize work in the scheduler's heap
                        with nc.named_scope(f"S{_pt}"):
                            sc = scp.tile([128, 2 * IC], FP32, tag="sc", name="sc")
                            for h in range(2):
                                nc.tensor.matmul(
                                    sc[:, h * IC : (h + 1) * IC],
                                    k_t[h * DH : (h + 1) * DH, p, jt * 128 : (jt + 1) * 128],
                                    q_t[h * DH : (h + 1) * DH, p, i0 : i0 + IC],
                                    start=True, stop=True,
                                )
                            nc.scalar.activation(
                                out=at[:, jt % 6, :], in_=sc[:, :],
                                func=mybir.ActivationFunctionType.Exp,
                                scale=SCALE,
                            )
                        inject(_pt)
                        if jt >= 1:
                            with nc.named_scope(f"P{_pt}"):
                                emit_pv(jt - 1)
                    with nc.named_scope(f"Pend{p}_{ic}"):
                        emit_pv(JT - 1)

                    # normalization: denominator sits replicated in pv
                    # rows 64:128, so 1/den is one fast DVE op and the
                    # multiply doubles as the psum evacuation — no partition
                    # broadcast, no PE involvement, nothing on the exp path
                    for h in range(2):
                        densb = drainp.tile([DH, IC], FP32, tag="densb", name="densb")
                        nc.vector.tensor_copy(densb[:, :], pv[h][DH : 2 * DH, :])
                        rec = drainp.tile([DH, IC], FP32, tag="rec", name="rec")
                        nc.vector.reciprocal_approx_fast(rec[:, :], densb[:, :])
                        osb = outp.tile([DH, IC], FP32, tag="osb", name="osb")
                        nc.vector.tensor_mul(osb[:, :], pv[h][0:DH, :], rec[:, :])
                        nc.sync.dma_start(
                            out=out[
                                (2 * p + h) * DH : (2 * p + h + 1) * DH,
                                i0 : i0 + IC,
                            ],
                            in_=osb[:, :],
                        )

            emit_attention(0)
            emit_attention(1)
            for fn in pending_steps:
                fn()
            pending_steps.clear()

    nc.finalize()
    return nc


_PROGRAM_CACHE = {}


def _get_program(S_, B_):
    assert (S_, B_) == (S, B)
    if "p" not in _PROGRAM_CACHE:
        _PROGRAM_CACHE["p"] = build_program()
    return _PROGRAM_CACHE["p"]


def make_in_maps(query, key, value, Wq, bq, Wk, bk, Wv, bv):
    S_, B_, D_ = query.shape
    assert (S_, B_, D_) == (S, B, D)

    def xt(a, b):
        # [S, B, D] -> [D, S] for batch b -> tiles [NTB, 128, KT, TB]
        aT = np.asarray(a[:, b, :], np.float32).T
        a4 = aT.reshape(KT, 128, NTB, TB).transpose(2, 1, 0, 3)
        return np.ascontiguousarray(a4).astype(NP_BF16)

    def wt_host(W, rows):
        # [DC rows, D] slice -> W.T [D, DC] -> [128, KT, DC] (partition-major)
        wT = np.asarray(W)[rows, :].T.astype(np.float32)
        w3 = wT.reshape(KT, 128, DC).transpose(1, 0, 2)
        return np.ascontiguousarray(w3).astype(NP_BF16)

    xq_b = [xt(query, b) for b in range(B)]
    xk_b = [xt(key, b) for b in range(B)]
    xv_b = [xt(value, b) for b in range(B)]

    in_maps = []
    for c in range(NCORES):
        b, hg = c // 4, c % 4
        rows = slice(hg * DC, (hg + 1) * DC)
        in_maps.append(
            {
                "xq": xq_b[b], "xk": xk_b[b], "xv": xv_b[b],
                "wq": wt_host(Wq, rows),
                "wk": wt_host(Wk, rows),
                "wv": wt_host(Wv, rows),
                "bqkv": np.ascontiguousarray(
                    np.stack(
                        [np.asarray(bq)[rows], np.asarray(bk)[rows], np.asarray(bv)[rows]],
                        axis=1,
                    ).reshape(NPAIR, 128, 3).transpose(1, 0, 2)
                ).astype(np.float32),
            }
        )
    return in_maps


def gather_output(results, S_, B_):
    full = np.empty((S, B, D), np.float32)
    for c in range(NCORES):
        b, hg = c // 4, c % 4
        o = np.asarray(results[c]["out"], np.float32)  # [DC, S]
        full[:, b, hg * DC : (hg + 1) * DC] = o.T
    return full


def kernel(query, key, value, Wq, bq, Wk, bk, Wv, bv):
    from concourse.bass_utils import run_bass_kernel_spmd

    S_, B_, _ = query.shape
    nc = _get_program(S_, B_)
    in_maps = make_in_maps(query, key, value, Wq, bq, Wk, bk, Wv, bv)
    res = run_bass_kernel_spmd(nc, in_maps, list(range(NCORES)))
    return gather_output(res.results, S_, B_)


# revision 5
# speedup vs baseline: 1.0357x; 1.0182x over previous
"""MultiHeadAttention kernel for Trainium2, 8-core hybrid batch x head sharding.

Problem: S=2048, B=2, D=1024, 16 heads of d=64 (batch_first=False).
Sharding: core c handles batch b=c//4 and head group hg=c%4 (4 heads =
256 output dims), processed as 2 "pairs" of 2 heads (a pair = 128
partitions = 2x64 head dims). Each core reads only its batch's
activations (12MB instead of 24MB) plus its 256-column weight slices.

Per-core dataflow:
  q^T, k^T [128, S] per pair = W_pair @ x^T            (PE, bf16, fp32 psum)
  v^T      [128, S] likewise -> DMA-xbar transpose to token-major
           v' [tok, jt, head, 128] with 64 ones columns (softmax denom)
  scores   per j-tile: both heads into ONE [128, 1024] psum tile
           (h0 cols 0:512, h1 cols 512:1024), K=64 matmuls at row
           positions 0/64 (row-packable)
  attn     = exp(scores * 1/8) in ONE ScalarE activation [128, 1024]
           (no max-subtract: scores*scale is small), bf16 out
  pv       [128, 512] per head += v'^T . attn, trailing the exps by one
           j-tile; v' cols 64:128 are ones, so pv rows 64:128 hold the
           softmax denominator replicated across partitions
  out      = pv[0:64] * recip(pv[64:128]): one SBUF staging copy, one
           reciprocal_approx_fast, one DVE multiply — no partition
           broadcast, no DMA round-trips, nothing on the PE
The exp stream is the critical engine (128 x [128,1024] activations
~= 128us); projections and v-transposes are deadline-scheduled into the
PE/DMA idle slack of the attention pipeline.
"""

import sys

if "/opt/trn_rl_repo" not in sys.path:
    sys.path.insert(0, "/opt/trn_rl_repo")

import numpy as np
import ml_dtypes

import concourse.bass as bass
import concourse.mybir as mybir
import concourse.tile as tile
from concourse import bacc

BF16 = mybir.dt.bfloat16
FP32 = mybir.dt.float32
NP_BF16 = ml_dtypes.bfloat16

D = 1024
NHEAD = 16
DH = 64
NCORES = 8
S = 2048
B = 2
HPC = 4                      # heads per core
DC = HPC * DH                # per-core output dims = 256
NPAIR = 2                    # head pairs per core (128 dims each)
KT = D // 128                # contraction tiles = 8
TB = 512                     # token block for projections
NTB = S // TB                # 4
IC = 512                     # i-chunk width
NIC = S // IC                # 4
JT = S // 128                # j-tiles = 16
SCALE = 1.0 / float(np.sqrt(DH))


def build_program():
    nc = bacc.Bacc(
        "TRN2", target_bir_lowering=False, debug=False, num_devices=NCORES
    )
    xq = nc.dram_tensor("xq", [NTB, 128, KT, TB], BF16, kind="ExternalInput")
    xk = nc.dram_tensor("xk", [NTB, 128, KT, TB], BF16, kind="ExternalInput")
    xv = nc.dram_tensor("xv", [NTB, 128, KT, TB], BF16, kind="ExternalInput")
    wq = nc.dram_tensor("wq", [128, KT, DC], BF16, kind="ExternalInput")
    wk = nc.dram_tensor("wk", [128, KT, DC], BF16, kind="ExternalInput")
    wv = nc.dram_tensor("wv", [128, KT, DC], BF16, kind="ExternalInput")
    bqkv = nc.dram_tensor("bqkv", [128, NPAIR, 3], FP32, kind="ExternalInput")
    out = nc.dram_tensor("out", [DC, S], FP32, kind="ExternalOutput")

    with tile.TileContext(nc) as tc:
        with (
            tc.tile_pool(name="const", bufs=1) as constp,
            tc.tile_pool(name="xin", bufs=1) as xp,
            tc.tile_pool(name="qkv", bufs=1) as qkvp,
            tc.tile_pool(name="vstg", bufs=2) as vstgp,
            tc.tile_pool(name="attn", bufs=2) as atp,
            tc.tile_pool(name="outp", bufs=2) as outp,
            tc.tile_pool(name="drain", bufs=2) as drainp,
            tc.tile_pool(name="sc", bufs=3, space="PSUM") as scp,
            tc.tile_pool(name="pv", bufs=2, space="PSUM") as pvp,
        ):
            # ---- input loads first, ALL on the SWDGE (gpsimd) queue which
            # starts draining earliest; interleaved so each projection's
            # weight arrives just before its x tile
            xq_t = xp.tile([128, NTB, KT, TB], BF16, tag="xq")
            xk_t = xp.tile([128, NTB, KT, TB], BF16, tag="xk")
            xv_t = xp.tile([128, NTB, KT, TB], BF16, tag="xv")
            xts = {0: xq_t, 1: xk_t, 2: xv_t}
            xsrc = {0: xq, 1: xk, 2: xv}
            wq_t = constp.tile([128, KT, DC], BF16, tag="wq")
            wk_t = constp.tile([128, KT, DC], BF16, tag="wk")
            wv_t = constp.tile([128, KT, DC], BF16, tag="wv")

            def dma_x(kind, tb):
                nc.gpsimd.dma_start(out=xts[kind][:, tb], in_=xsrc[kind][tb])

            nc.gpsimd.dma_start(out=wq_t[:], in_=wq[:, :, :])
            dma_x(0, 0)
            nc.gpsimd.dma_start(out=wk_t[:], in_=wk[:, :, :])
            dma_x(1, 0)
            nc.gpsimd.dma_start(out=wv_t[:], in_=wv[:, :, :])
            dma_x(2, 0)
            # the rest of the stream in consumption order; the queue drains
            # continuously at HBM rate, so emitting everything now is strictly
            # better than injecting starts later
            for kind, tb in [(1, 1), (2, 1), (1, 2), (2, 2), (1, 3), (2, 3),
                             (0, 1), (0, 2), (0, 3)]:
                dma_x(kind, tb)

            # ---- exp table preload: fire ACT_TABLE_LOAD during the DMA phase
            pre_in = constp.tile([128, 16], FP32, tag="prei")
            nc.vector.memset(pre_in[:], 0.0)
            pre_out = constp.tile([128, 16], BF16, tag="preo")
            nc.scalar.activation(
                out=pre_out[:], in_=pre_in[:],
                func=mybir.ActivationFunctionType.Exp, scale=1.0,
            )

            # ---- constants
            bqkv_t = constp.tile([128, NPAIR, 3], FP32, tag="bqkv")
            nc.sync.dma_start(out=bqkv_t[:], in_=bqkv[:, :, :])

            # ---- persistent activations
            q_t = qkvp.tile([128, NPAIR, S], BF16, tag="q")
            k_t = qkvp.tile([128, NPAIR, S], BF16, tag="k")
            # token-major projected v, two stages: vx_t is the contiguous
            # DMA-xbar transpose target [tok, jt, 128 dims]; v_t adds the
            # ones column per head (softmax denominator) via one cheap DVE
            # rearrange copy per projection unit.
            vx_t = qkvp.tile([128, NPAIR, JT, 2 * DH], BF16, tag="vx")
            # per head: cols 0:64 = projected v, cols 64:128 = ones, so the
            # pv matmul replicates the softmax denominator across output
            # partitions 64:128 (normalization then needs no partition
            # broadcast at all)
            v_t = qkvp.tile([128, NPAIR, JT, 2, 2 * DH], BF16, tag="v")
            nc.vector.memset(v_t[:, :, :, :, DH : 2 * DH], 1.0)
            # global attention-weights ring: 12 slots indexed by the global
            # j-tile point, so pv can trail the exps by up to 11 j-tiles
            at_t = atp.tile([128, 12, 2 * IC], BF16, tag="at", bufs=1)

            wts = {0: wq_t, 1: wk_t, 2: wv_t}

            # ---- HAM warmup: a dense identical-weights matmul burst chained
            # on the weights so the PE monitor reaches 8/8 before projections
            warm = scp.tile([128, 128], FP32, tag="sc", name="warm")
            for _ in range(24):
                nc.tensor.matmul(
                    warm[:, :], wq_t[:, 0, 0:128], wq_t[:, 0, 0:128],
                    start=True, stop=True,
                )

            proj_ps = {}

            def emit_proj_mm(kind, tb, p, half):
                # one half of a projection's K-accumulation: 4 matmuls ~0.9us
                x_t = xts[kind]
                w_t = wts[kind]
                if half == 0:
                    # projection scratch shares the (triple-buffered) scores
                    # tag: a proj ps occupies one slot for ~2 points while
                    # scores keep double-buffering through the other two
                    proj_ps[(kind, tb, p)] = scp.tile(
                        [128, TB], FP32, tag="sc", name=f"ps{kind}{tb}{p}"
                    )
                ps = proj_ps[(kind, tb, p)]
                for kt in range(half * 4, half * 4 + 4):
                    nc.tensor.matmul(
                        ps[:, :], w_t[:, kt, p * 128 : (p + 1) * 128],
                        x_t[:, tb, kt, :],
                        start=(kt == 0), stop=(kt == KT - 1),
                    )
                if half == 1:
                    bias = bqkv_t[:, p, kind : kind + 1].to_broadcast((128, TB))
                    if kind < 2:
                        dst = q_t if kind == 0 else k_t
                        nc.vector.tensor_add(
                            dst[:, p, tb * TB : (tb + 1) * TB], ps[:, :], bias
                        )
                    else:
                        vTt = vstgp.tile([128, TB], BF16, tag="vT", name="vT")
                        nc.vector.tensor_add(vTt[:, :], ps[:, :], bias)
                        j0 = tb * (TB // 128)
                        j1 = (tb + 1) * (TB // 128)
                        nc.sync.dma_start_transpose(vx_t[:, p, j0:j1, :], vTt[:, :])
                        nc.vector.tensor_copy(
                            v_t[:, p, j0:j1, :, 0:DH],
                            vx_t[:, p, j0:j1, :].rearrange("t j (h d) -> t j h d", h=2),
                        )
                    del proj_ps[(kind, tb, p)]

            def emit_proj(kind, tb, p):
                emit_proj_mm(kind, tb, p, 0)
                emit_proj_mm(kind, tb, p, 1)

            # prologue: just enough for p0's chunk-0 to start (j-tiles 0-3
            # only need k/v block 0); the rest injects at deadline points
            emit_proj(0, 0, 0)
            emit_proj(1, 0, 0)
            emit_proj(2, 0, 0)

            # ---- deadline-scheduled units (half-projection granularity so no
            # single injection holds the PE for long).
            # point = (p*NIC + ic)*JT + jt.
            # p0 deadlines (finish by): k(tb) 4*tb, v(tb) 4*tb+1, q(tb) 16*tb.
            # p1 deadlines: +64 on everything.
            def _scoped(name, fn):
                def g():
                    with nc.named_scope(name):
                        fn()
                return g

            def proj_halves(pt0, pt1, kind, tb, p):
                n = f"U{'qkv'[kind]}{tb}p{p}"
                return [
                    (pt0, _scoped(n + "a", lambda: emit_proj_mm(kind, tb, p, 0))),
                    (pt1, _scoped(n + "b", lambda: emit_proj_mm(kind, tb, p, 1))),
                ]

            units = (
                proj_halves(1, 2, 1, 1, 0)
                + proj_halves(2, 3, 2, 1, 0)
                + proj_halves(4, 5, 1, 2, 0)
                + proj_halves(6, 7, 2, 2, 0)
                + proj_halves(8, 9, 1, 3, 0)
                + proj_halves(10, 11, 2, 3, 0)
                + proj_halves(13, 14, 0, 1, 0)
                + proj_halves(18, 20, 0, 2, 0)
                + proj_halves(24, 26, 0, 3, 0)
                + proj_halves(28, 30, 1, 0, 1)
                + proj_halves(32, 34, 2, 0, 1)
                + proj_halves(36, 38, 1, 1, 1)
                + proj_halves(40, 42, 2, 1, 1)
                + proj_halves(44, 46, 1, 2, 1)
                + proj_halves(48, 50, 2, 2, 1)
                + proj_halves(52, 54, 1, 3, 1)
                + proj_halves(56, 58, 2, 3, 1)
                + proj_halves(60, 62, 0, 0, 1)
                + proj_halves(68, 70, 0, 1, 1)
                + proj_halves(84, 86, 0, 2, 1)
                + proj_halves(100, 102, 0, 3, 1)
            )
            units.sort(key=lambda u: u[0])
            ui = [0]
            pending_steps = []

            def inject(point):
                while ui[0] < len(units) and units[ui[0]][0] <= point:
                    units[ui[0]][1]()
                    ui[0] += 1
                if pending_steps:
                    pending_steps.pop(0)()

            def emit_attention(p):
                for ic in range(NIC):
                    i0 = ic * IC
                    pv = [
                        pvp.tile([128, IC], FP32, tag="pv", name=f"pv{h}")
                        for h in range(2)
                    ]
                    base = (p * NIC + ic) * JT

                    def emit_pv(j):
                        for h in range(2):
                            nc.tensor.matmul(
                                pv[h][:, :],
                                v_t[:, p, j, h, :],
                                at_t[:, (base + j) % 12, h * IC : (h + 1) * IC],
                                start=(j == 0), stop=(j == JT - 1),
                            )

                    for jt in range(JT):
                        # scores + exp FIRST: they are the ACT-critical chain,
                        # and the PE queue is in-order — anything emitted
                        # before sc(jt) delays the next exp by its duration
                        _pt = (p * NIC + ic) * JT + jt
                        # high_priority: the scores->exp chain is the critical
                        # path; this lets a ready score matmul preempt queued
                        # projection/finalize work in the scheduler's heap
                        with nc.named_scope(f"S{_pt}"):
                            sc = scp.tile([128, 2 * IC], FP32, tag="sc", name="sc")
                            for h in range(2):
                                nc.tensor.matmul(
                                    sc[:, h * IC : (h + 1) * IC],
                                    k_t[h * DH : (h + 1) * DH, p, jt * 128 : (jt + 1) * 128],
                                    q_t[h * DH : (h + 1) * DH, p, i0 : i0 + IC],
                                    start=True, stop=True,
                                )
                            nc.scalar.activation(
                                out=at_t[:, (base + jt) % 12, :], in_=sc[:, :],
                                func=mybir.ActivationFunctionType.Exp,
                                scale=SCALE,
                            )
                        inject(_pt)
                        if jt >= 1:
                            with nc.named_scope(f"P{_pt}"):
                                emit_pv(jt - 1)
                    with nc.named_scope(f"Pend{p}_{ic}"):
                        emit_pv(JT - 1)

                    # normalization: denominator sits replicated in pv
                    # rows 64:128, so 1/den is one fast DVE op and the
                    # multiply doubles as the psum evacuation — no partition
                    # broadcast, no PE involvement, nothing on the exp path
                    for h in range(2):
                        densb = drainp.tile([DH, IC], FP32, tag="densb", name="densb")
                        nc.vector.tensor_copy(densb[:, :], pv[h][DH : 2 * DH, :])
                        rec = drainp.tile([DH, IC], FP32, tag="rec", name="rec")
                        nc.vector.reciprocal_approx_fast(rec[:, :], densb[:, :])
                        osb = outp.tile([DH, IC], FP32, tag="osb", name="osb")
                        nc.vector.tensor_mul(osb[:, :], pv[h][0:DH, :], rec[:, :])
                        nc.sync.dma_start(
                            out=out[
                                (2 * p + h) * DH : (2 * p + h + 1) * DH,
                                i0 : i0 + IC,
                            ],
                            in_=osb[:, :],
                        )

            emit_attention(0)
            emit_attention(1)
            for fn in pending_steps:
                fn()
            pending_steps.clear()

    nc.finalize()
    return nc


_PROGRAM_CACHE = {}


def _get_program(S_, B_):
    assert (S_, B_) == (S, B)
    if "p" not in _PROGRAM_CACHE:
        _PROGRAM_CACHE["p"] = build_program()
    return _PROGRAM_CACHE["p"]


def make_in_maps(query, key, value, Wq, bq, Wk, bk, Wv, bv):
    S_, B_, D_ = query.shape
    assert (S_, B_, D_) == (S, B, D)

    def xt(a, b):
        # [S, B, D] -> [D, S] for batch b -> tiles [NTB, 128, KT, TB]
        aT = np.asarray(a[:, b, :], np.float32).T
        a4 = aT.reshape(KT, 128, NTB, TB).transpose(2, 1, 0, 3)
        return np.ascontiguousarray(a4).astype(NP_BF16)

    def wt_host(W, rows):
        # [DC rows, D] slice -> W.T [D, DC] -> [128, KT, DC] (partition-major)
        wT = np.asarray(W)[rows, :].T.astype(np.float32)
        w3 = wT.reshape(KT, 128, DC).transpose(1, 0, 2)
        return np.ascontiguousarray(w3).astype(NP_BF16)

    xq_b = [xt(query, b) for b in range(B)]
    xk_b = [xt(key, b) for b in range(B)]
    xv_b = [xt(value, b) for b in range(B)]

    in_maps = []
    for c in range(NCORES):
        b, hg = c // 4, c % 4
        rows = slice(hg * DC, (hg + 1) * DC)
        in_maps.append(
            {
                "xq": xq_b[b], "xk": xk_b[b], "xv": xv_b[b],
                "wq": wt_host(Wq, rows),
                "wk": wt_host(Wk, rows),
                "wv": wt_host(Wv, rows),
                "bqkv": np.ascontiguousarray(
                    np.stack(
                        [np.asarray(bq)[rows], np.asarray(bk)[rows], np.asarray(bv)[rows]],
                        axis=1,
                    ).reshape(NPAIR, 128, 3).transpose(1, 0, 2)
                ).astype(np.float32),
            }
        )
    return in_maps


def gather_output(results, S_, B_):
    full = np.empty((S, B, D), np.float32)
    for c in range(NCORES):
        b, hg = c // 4, c % 4
        o = np.asarray(results[c]["out"], np.float32)  # [DC, S]
        full[:, b, hg * DC : (hg + 1) * DC] = o.T
    return full


def kernel(query, key, value, Wq, bq, Wk, bk, Wv, bv):
    from concourse.bass_utils import run_bass_kernel_spmd

    S_, B_, _ = query.shape
    nc = _get_program(S_, B_)
    in_maps = make_in_maps(query, key, value, Wq, bq, Wk, bk, Wv, bv)
    res = run_bass_kernel_spmd(nc, in_maps, list(range(NCORES)))
    return gather_output(res.results, S_, B_)
